# revision 6
# baseline (speedup 1.0000x reference)
"""Trainium2 Bass kernel for CompositionalResidualMLP (MoE routing, 2-node module network).

Strategy: data-parallel over batch across 8 NeuronCores. On the host, samples are
dealt round-robin (per routing pair) to cores and sorted into 64 (a0,a1) blocks
per core with per-a1 block capacities, so every layer is a dense per-module
matmul. Activations live feature-major in SBUF ([features, samples]); feat0 and
feat1 are stacked on partition halves of one input image so node-0 layers read
rows 0:64 and node-1 pre layers read rows 64:128.

Perf notes vs the first working version:
- No warm-up matmul burst; instead a short burst of tiny junk matmuls runs
  while input DMAs stream so the PE HAM clock-gate flips to 8/8 early.
- DMA dispatches (DIRECT2D, ~600ns each on the issuing queue) are spread
  across sync/tensor/vector/gpsimd so input transfers start ~immediately.
- PSUM->SBUF evictions are the throughput wall (1 elem/cycle/lane from PSUM
  on both ACT and DVE): they strictly alternate between the two engines and
  the schedule interleaves eviction-light layers (L2/L4) with eviction-heavy
  ones (L1/L3) so the PE never idles long enough to re-throttle.
- L5 (final, 32-wide) outputs for 4 modules are packed into partition
  quarters of one PSUM tile via tile_position, so 8 narrow evictions become
  2 full-width copies; its bias is added on the host after the gather.
- A dummy activation at t=0 pulls the one-time ~1.3us ACT_TABLE_LOAD off the
  critical path.
"""

import numpy as np
from contextlib import ExitStack

# Problem constants (hardcoded per contract)
B_TOT = 32768
D0 = 64
D1 = 64
M = 8          # modules per node
H = 256        # hidden width
O0 = 128
O1 = 32
NCORES = 8

QUARTET = True   # pack 4 L5 modules per PSUM tile via tile_position
SPAM_MMS = 16    # tiny junk matmuls at start (HAM clock-gate warm-up)


def _build_bass(S, Sp, Cj, off, Cmax):
    """Per-core Bass program; all cores run the identical program (pure SPMD)."""
    import concourse.bass as bass
    import concourse.tile as tile
    from concourse import bacc, mybir

    F32 = mybir.dt.float32
    BF16 = mybir.dt.bfloat16
    Relu = mybir.ActivationFunctionType.Relu
    Add = mybir.AluOpType.add
    Max = mybir.AluOpType.max

    ncolp = M * Sp
    half = S // 2
    Q = 2 * 4 * Cmax  # one quartet's output image width
    assert S % 4 == 0 and S <= 1024 and Cmax <= 128
    assert all(off[j] + Cmax <= Sp for j in range(M))

    nc = bacc.Bacc("TRN2", target_bir_lowering=False, debug=False,
                   enable_asserts=False, num_devices=NCORES)

    # DRAM I/O
    xt = nc.dram_tensor("xt", [128, ncolp], BF16, kind="ExternalInput").ap()
    wab = nc.dram_tensor("wab", [128, M * H], BF16, kind="ExternalInput").ap()
    w01 = nc.dram_tensor("w01", [128, 2 * M * O0], BF16, kind="ExternalInput").ap()
    wc = nc.dram_tensor("wc", [128, 3 * M * O0 + M * O1], BF16, kind="ExternalInput").ap()
    bsd = nc.dram_tensor("bs", [128, 48], F32, kind="ExternalInput").ap()
    outT = nc.dram_tensor("outT", [128, 2 * Q], BF16, kind="ExternalOutput").ap()

    with tile.TileContext(nc) as tc:
        with ExitStack() as ctx:
            acts = ctx.enter_context(tc.tile_pool(name="acts", bufs=1))
            wpool = ctx.enter_context(tc.tile_pool(name="w", bufs=1))
            ps = ctx.enter_context(tc.tile_pool(name="ps", bufs=3, space="PSUM"))
            psq = ctx.enter_context(tc.tile_pool(name="psq", bufs=1, space="PSUM"))

            wu = wpool.tile([64, 64], BF16, tag="wu")
            dumt = wpool.tile([1, 2], F32, tag="dumt")
            wABs = wpool.tile([128, M * H], BF16, tag="wab")
            w01s = wpool.tile([128, 2 * M * O0], BF16, tag="w01")
            wCs = wpool.tile([128, 3 * M * O0 + M * O1], BF16, tag="wc")
            bss = wpool.tile([128, 48], F32, tag="bs")

            xts = acts.tile([128, ncolp], BF16, tag="xt")
            h1a = acts.tile([128, ncolp], BF16, tag="h1a")
            h1b = acts.tile([128, ncolp], BF16, tag="h1b")
            hs = acts.tile([128, ncolp], BF16, tag="h")
            g1a = acts.tile([128, ncolp], BF16, tag="g1a")
            g1b = acts.tile([128, ncolp], BF16, tag="g1b")
            gs = acts.tile([128, ncolp], BF16, tag="gs")
            outs0 = acts.tile([128, Q], BF16, tag="out0")
            outs1 = acts.tile([128, Q], BF16, tag="out1")
            outs = [outs0, outs1]

            # ---- head: spread DMA dispatch over engines; warm ACT table + PE
            nc.vector.memset(wu[:], 0.0)
            nc.gpsimd.dma_start(bss[:], bsd)
            nc.gpsimd.dma_start(wABs[:], wab)
            nc.sync.dma_start(xts[:, 0:4 * Sp], xt[:, 0:4 * Sp])
            nc.scalar.dma_start(xts[:, 4 * Sp:ncolp], xt[:, 4 * Sp:ncolp])
            nc.scalar.dma_start(w01s[:], w01)
            nc.scalar.activation(dumt[:], wu[0:1, 0:2], Relu)
            nc.gpsimd.dma_start(wCs[:], wc)
            if Sp > S:
                gsv = gs[:].rearrange("p (i c) -> p i c", i=M)
                nc.gpsimd.memset(gsv[:, :, S:Sp], 0.0)

            ptspam = psq.tile([128, 1024], F32, tag="q")
            for _ in range(SPAM_MMS):
                nc.tensor.matmul(ptspam[0:64, 0:64], wu[:], wu[:],
                                 start=True, stop=True)

            parity = 0

            def evict_relu(dst_ap, src_ap, bias_ap):
                nonlocal parity
                if parity == 0:
                    nc.scalar.activation(dst_ap, src_ap, Relu, bias=bias_ap)
                else:
                    nc.vector.tensor_scalar(dst_ap, src_ap, bias_ap, 0.0, Add, Max)
                parity ^= 1

            def strided(tile_t, j, b0, nb, c):
                v = tile_t[:].rearrange("p (i c) -> p i c", i=M)
                return v[:, b0:b0 + nb, off[j]:off[j] + c]

            # ---- L1: h1 = relu(W00[a0].T @ x0 + b00[a0])   (K=64, rows 0:64)
            def L1(i):
                for mo in range(2):
                    pt = ps.tile([128, 1024], F32, tag="ps")
                    ptb = pt[:].rearrange("p (b c) -> p b c", b=2)
                    for s in range(2):
                        nc.tensor.matmul(
                            ptb[:, s, 0:half],
                            wABs[0:64, i * H + mo * 128: i * H + (mo + 1) * 128],
                            xts[0:64, i * Sp + s * half: i * Sp + (s + 1) * half],
                            start=True, stop=True)
                    dst_tile = h1a if mo == 0 else h1b
                    dst = dst_tile[:, i * Sp: i * Sp + S].rearrange("p (b c) -> p b c", b=2)
                    evict_relu(dst, ptb[:, :, 0:half], bss[:, mo * M + i: mo * M + i + 1])

            # ---- L2: h = relu(W01[a0].T @ h1 + b01[a0])   (K=256 -> 2 accum)
            def L2(i):
                pt = ps.tile([128, 1024], F32, tag="ps")
                ptb = pt[:].rearrange("p (b c) -> p b c", b=2)
                for s in range(2):
                    for kc, srcT in enumerate((h1a, h1b)):
                        nc.tensor.matmul(
                            ptb[:, s, 0:half],
                            w01s[:, (kc * M + i) * O0: (kc * M + i + 1) * O0],
                            srcT[:, i * Sp + s * half: i * Sp + (s + 1) * half],
                            start=(kc == 0), stop=(kc == 1))
                dst = hs[:, i * Sp: i * Sp + S].rearrange("p (b c) -> p b c", b=2)
                evict_relu(dst, ptb[:, :, 0:half], bss[:, 16 + i: 16 + i + 1])

            # ---- L3: g1 = relu(W1p[a1].T @ x1 + b1p[a1])  (K=64, rows 64:128)
            def L3(j):
                cj = Cj[j]
                xv = xts[:].rearrange("p (i c) -> p i c", i=M)
                for mo in range(2):
                    pt = ps.tile([128, 1024], F32, tag="ps")
                    ptb = pt[:].rearrange("p (b c) -> p b c", b=2)
                    for s in range(2):
                        nc.tensor.matmul(
                            ptb[:, s, 0:4 * cj].rearrange("p (i c) -> p i c", c=cj),
                            wABs[64:128, j * H + mo * 128: j * H + (mo + 1) * 128],
                            xv[64:128, 4 * s:4 * s + 4, off[j]:off[j] + cj],
                            start=True, stop=True)
                    src_ap = ptb[:, :, 0:4 * cj].rearrange("p b (i c) -> p b i c", c=cj)
                    dst_tile = g1a if mo == 0 else g1b
                    evict_relu(strided(dst_tile, j, 0, 8, cj), src_ap,
                               bss[:, 24 + mo * M + j: 24 + mo * M + j + 1])

            # ---- L4: g = relu(W1a[a1].T @ concat(h, g1) + b1a[a1]) (K=384 -> 3 accum)
            def L4(j):
                cj = Cj[j]
                pt = ps.tile([128, 1024], F32, tag="ps")
                ptb = pt[:].rearrange("p (b c) -> p b c", b=2)
                for s in range(2):
                    ptv = ptb[:, s, 0:4 * cj].rearrange("p (i c) -> p i c", c=cj)
                    for kc, srcT in enumerate((hs, g1a, g1b)):
                        nc.tensor.matmul(
                            ptv,
                            wCs[:, (kc * M + j) * O0: (kc * M + j + 1) * O0],
                            strided(srcT, j, 4 * s, 4, cj),
                            start=(kc == 0), stop=(kc == 2))
                src_ap = ptb[:, :, 0:4 * cj].rearrange("p b (i c) -> p b i c", c=cj)
                evict_relu(strided(gs, j, 0, 8, cj), src_ap,
                           bss[:, 40 + j: 40 + j + 1])

            # ---- L5: out = W1o[a1].T @ g  (identity; bias added on host)
            # 4 modules land in partition quarters of one PSUM tile.
            def L5(j, ptq):
                r = j % 4
                ptb = ptq[:].rearrange("p (b c) -> p b c", b=2)
                for s in range(2):
                    out_ap = ptb[32 * r:32 * r + 32, s, 0:4 * Cmax].rearrange(
                        "p (i c) -> p i c", c=Cmax)
                    nc.tensor.matmul(
                        out_ap,
                        wCs[:, 3 * M * O0 + j * O1: 3 * M * O0 + (j + 1) * O1],
                        strided(gs, j, 4 * s, 4, Cmax),
                        start=True, stop=True,
                        tile_position=(0, 32 * r))

            def evict_quartet(q, ptq):
                ptb = ptq[:].rearrange("p (b c) -> p b c", b=2)
                dst = outs[q][:].rearrange("p (b c) -> p b c", b=2)
                if q == 0:
                    nc.scalar.copy(dst, ptb[:, :, 0:4 * Cmax])
                    nc.gpsimd.dma_start(outT[:, 0:Q], outs[q][:])
                else:
                    nc.vector.tensor_copy(dst, ptb[:, :, 0:4 * Cmax])
                    nc.sync.dma_start(outT[:, Q:2 * Q], outs[q][:])

            # ---- schedule: L1/L2 interleaved, then per-j L3+L4 with lagged L5
            L1(0)
            L1(1)
            for i in range(2, M):
                L2(i - 2)
                L1(i)
            L2(M - 2)
            L2(M - 1)

            ptq = None
            for j in range(M):
                L3(j)
                L4(j)
                if j >= 1:
                    jj = j - 1
                    if jj % 4 == 0:
                        ptq = psq.tile([128, 1024], F32, tag="q")
                    L5(jj, ptq)
                    if jj == 3:
                        evict_quartet(0, ptq)
            L5(7, ptq)
            evict_quartet(1, ptq)

    nc.compile()
    return nc


def _pack_inputs(inputs, core, xcol, S, Sp, Cj, off, Cmax):
    """Host-side packing of weights/biases/inputs into SBUF-image layouts."""
    import ml_dtypes
    bf = ml_dtypes.bfloat16
    f = lambda a: np.ascontiguousarray(a.astype(bf))
    g = lambda a: np.ascontiguousarray(a.astype(np.float32))
    W00 = inputs["W00"]; W01 = inputs["W01"]; W1p = inputs["W1p"]
    W1a = inputs["W1a"]; W1o = inputs["W1o"]

    wab = np.concatenate([
        W00.transpose(1, 0, 2).reshape(D0, M * H),
        W1p.transpose(1, 0, 2).reshape(D1, M * H)], axis=0)
    w01 = W01.reshape(M, 2, 128, O0).transpose(2, 1, 0, 3).reshape(128, 2 * M * O0)
    wc = np.concatenate([
        W1a.reshape(M, 3, 128, O0).transpose(2, 1, 0, 3).reshape(128, 3 * M * O0),
        W1o.transpose(1, 0, 2).reshape(128, M * O1)], axis=1)

    bs = np.zeros((128, 48), dtype=np.float32)
    bs[:, 0:16] = inputs["b00"].reshape(M, 2, 128).transpose(2, 1, 0).reshape(128, 16)
    bs[:, 16:24] = inputs["b01"].T
    bs[:, 24:40] = inputs["b1p"].reshape(M, 2, 128).transpose(2, 1, 0).reshape(128, 16)
    bs[:, 40:48] = inputs["b1a"].T

    input_val = inputs["input_val"]
    feat0 = input_val[:, :D0].astype(bf)
    feat1 = input_val[:, D0:D0 + D1].astype(bf)
    ncolp = M * Sp
    XT = np.zeros((NCORES, 128, ncolp), dtype=bf)
    XT[core, 0:64, xcol] = feat0
    XT[core, 64:128, xcol] = feat1

    return {"wab": f(wab), "w01": f(w01), "wc": f(wc), "bs": g(bs)}, XT


def _route(input_val):
    """Assign each sample to a (core, column) in the blocked layout."""
    a0 = np.argmax(input_val[:, D0 + D1: D0 + D1 + M], axis=1)
    a1 = np.argmax(input_val[:, D0 + D1 + M: D0 + D1 + 2 * M], axis=1)
    B = input_val.shape[0]
    nij = np.zeros((M, M), dtype=np.int64)
    np.add.at(nij, (a0, a1), 1)
    # capacities rounded up to even, min 64
    Cj = np.maximum((-(-nij.max(axis=0) // NCORES) + 1) // 2 * 2, 64)
    off = np.concatenate([[0], np.cumsum(Cj)[:-1]]).astype(np.int64)
    S = int(Cj.sum())
    Cmax = int(Cj.max())
    # per-module stride padded so L5's uniform Cmax-wide reads stay in bounds
    pad = max(0, Cmax + int(off[-1]) - S)
    Sp = S + ((pad + 7) // 8) * 8

    pairkey = a0 * M + a1
    order = np.argsort(pairkey, kind="stable")
    counts = np.bincount(pairkey, minlength=M * M)
    group_start = np.concatenate([[0], np.cumsum(counts)[:-1]])
    rank_sorted = np.arange(B) - np.repeat(group_start, counts)
    rank = np.empty(B, dtype=np.int64)
    rank[order] = rank_sorted
    core = rank % NCORES
    slot = rank // NCORES
    assert np.all(slot < Cj[a1]), "capacity overflow"
    xcol = a0 * Sp + off[a1] + slot
    return (core, xcol, a0, a1, slot, S, Sp,
            [int(c) for c in Cj], [int(o) for o in off], Cmax)


def kernel(**inputs):
    import os
    from concourse.bass_utils import run_bass_kernel_spmd

    inputs = {k: np.asarray(v, dtype=np.float32) for k, v in inputs.items()}
    input_val = inputs["input_val"]

    core, xcol, a0, a1, slot, S, Sp, Cj, off, Cmax = _route(input_val)
    wmap, XT = _pack_inputs(inputs, core, xcol, S, Sp, Cj, off, Cmax)

    nc = _build_bass(S, Sp, Cj, off, Cmax)

    in_maps = [dict(wmap, xt=np.ascontiguousarray(XT[c])) for c in range(NCORES)]
    res = run_bass_kernel_spmd(nc, in_maps, core_ids=list(range(NCORES)),
                               tmpdir=os.environ.get("BASS_TMPDIR"))
    global _LAST_RESULTS
    _LAST_RESULTS = res

    Q = 2 * 4 * Cmax
    OUT = np.stack([r["outT"] for r in res.results]).astype(np.float32)  # [NC, 128, 2Q]
    rows = 32 * (a1 % 4)
    cols = (a1 // 4) * Q + (a0 // 4) * (4 * Cmax) + (a0 % 4) * Cmax + slot
    out = OUT[core[:, None], rows[:, None] + np.arange(O1)[None, :], cols[:, None]]
    out = out + inputs["b1o"][a1]
    return np.ascontiguousarray(out).astype(np.float32)


# revision 7
# speedup vs baseline: 1.3198x; 1.3198x over previous
"""Trainium2 Bass kernel for CompositionalResidualMLP (MoE routing, 2-node module network).

Strategy: data-parallel over batch across 8 NeuronCores. On the host, samples are
dealt round-robin (per routing pair) to cores and sorted into 64 (a0,a1) blocks
per core with per-a1 block capacities, so every layer is a dense per-module
matmul. Activations live feature-major in SBUF ([features, samples]); feat0 and
feat1 are stacked on partition halves of one input image so node-0 layers read
rows 0:64 and node-1 pre layers read rows 64:128.

Perf notes vs the first working version:
- No warm-up matmul burst; instead a short burst of tiny junk matmuls runs
  while input DMAs stream so the PE HAM clock-gate flips to 8/8 early.
- DMA dispatches (DIRECT2D, ~600ns each on the issuing queue) are spread
  across sync/tensor/vector/gpsimd so input transfers start ~immediately.
- PSUM->SBUF evictions are the throughput wall (1 elem/cycle/lane from PSUM
  on both ACT and DVE): they strictly alternate between the two engines and
  the schedule interleaves eviction-light layers (L2/L4) with eviction-heavy
  ones (L1/L3) so the PE never idles long enough to re-throttle.
- L5 (final, 32-wide) outputs for 4 modules are packed into partition
  quarters of one PSUM tile via tile_position, so 8 narrow evictions become
  2 full-width copies; its bias is added on the host after the gather.
- A dummy activation at t=0 pulls the one-time ~1.3us ACT_TABLE_LOAD off the
  critical path.
"""

import numpy as np
from contextlib import ExitStack

# Problem constants (hardcoded per contract)
B_TOT = 32768
D0 = 64
D1 = 64
M = 8          # modules per node
H = 256        # hidden width
O0 = 128
O1 = 32
NCORES = 8

QUARTET = True   # pack 4 L5 modules per PSUM tile via tile_position
SPAM_MMS = 10    # tiny junk matmuls at start (HAM clock-gate warm-up)


def _build_bass(S, Sp, Cj, off, Cmax):
    """Per-core Bass program; all cores run the identical program (pure SPMD)."""
    import concourse.bass as bass
    import concourse.tile as tile
    from concourse import bacc, mybir

    F32 = mybir.dt.float32
    BF16 = mybir.dt.bfloat16
    Relu = mybir.ActivationFunctionType.Relu
    Add = mybir.AluOpType.add
    Max = mybir.AluOpType.max

    ncolp = M * Sp
    half = S // 2
    Q = 2 * 4 * Cmax  # one quartet's output image width
    assert S % 4 == 0 and S <= 1024 and Cmax <= 128
    assert all(off[j] + Cmax <= Sp for j in range(M))

    nc = bacc.Bacc("TRN2", target_bir_lowering=False, debug=False,
                   enable_asserts=False, num_devices=NCORES)

    # DRAM I/O
    xt = nc.dram_tensor("xt", [128, ncolp], BF16, kind="ExternalInput").ap()
    w0z = nc.dram_tensor("w0z", [128, M * H], BF16, kind="ExternalInput").ap()
    w1z = nc.dram_tensor("w1z", [128, M * H], BF16, kind="ExternalInput").ap()
    w01 = nc.dram_tensor("w01", [128, 2 * M * O0], BF16, kind="ExternalInput").ap()
    wc = nc.dram_tensor("wc", [128, 3 * M * O0 + M * O1], BF16, kind="ExternalInput").ap()
    bsd = nc.dram_tensor("bs", [128, 48], F32, kind="ExternalInput").ap()
    outT = nc.dram_tensor("outT", [128, 2 * Q], BF16, kind="ExternalOutput").ap()

    with tile.TileContext(nc) as tc:
        with ExitStack() as ctx:
            acts = ctx.enter_context(tc.tile_pool(name="acts", bufs=1))
            wpool = ctx.enter_context(tc.tile_pool(name="w", bufs=1))
            ps = ctx.enter_context(tc.tile_pool(name="ps", bufs=3, space="PSUM"))
            psq = ctx.enter_context(tc.tile_pool(name="psq", bufs=1, space="PSUM"))

            wu = wpool.tile([128, 512], BF16, tag="wu")
            dumt = wpool.tile([1, 2], F32, tag="dumt")
            w0zs = wpool.tile([128, M * H], BF16, tag="w0z")
            w1zs = wpool.tile([128, M * H], BF16, tag="w1z")
            w01s = wpool.tile([128, 2 * M * O0], BF16, tag="w01")
            wCs = wpool.tile([128, 3 * M * O0 + M * O1], BF16, tag="wc")
            bss = wpool.tile([128, 48], F32, tag="bs")

            xts = acts.tile([128, ncolp], BF16, tag="xt")
            h1a = acts.tile([128, ncolp], BF16, tag="h1a")
            h1b = acts.tile([128, ncolp], BF16, tag="h1b")
            hs = acts.tile([128, ncolp], BF16, tag="h")
            g1a = acts.tile([128, ncolp], BF16, tag="g1a")
            g1b = acts.tile([128, ncolp], BF16, tag="g1b")
            gs = acts.tile([128, ncolp], BF16, tag="gs")
            outs0 = acts.tile([128, Q], BF16, tag="out0")
            outs1 = acts.tile([128, Q], BF16, tag="out1")
            outs = [outs0, outs1]

            # ---- head: spread DMA dispatch over engines; warm ACT table + PE
            nc.vector.memset(wu[:], 0.0)
            nc.sync.dma_start(bss[:], bsd)
            nc.sync.dma_start(w0zs[:], w0z)
            nc.sync.dma_start(xts[:, 0:4 * Sp], xt[:, 0:4 * Sp])
            nc.sync.dma_start(xts[:, 4 * Sp:ncolp], xt[:, 4 * Sp:ncolp])
            nc.sync.dma_start(w01s[:], w01)
            nc.sync.dma_start(w1zs[:], w1z)
            nc.sync.dma_start(wCs[:], wc)
            nc.scalar.activation(dumt[:], wu[0:1, 0:2], Relu)
            if Sp > S:
                gsv = gs[:].rearrange("p (i c) -> p i c", i=M)
                nc.gpsimd.memset(gsv[:, :, S:Sp], 0.0)

            ptspam = psq.tile([128, 1024], F32, tag="q")
            for _ in range(SPAM_MMS):
                nc.tensor.matmul(ptspam[:, 0:512], wu[:, 0:128], wu[:],
                                 start=True, stop=True)

            parity = 0

            def evict_relu(dst_ap, src_ap, bias_ap):
                nonlocal parity
                if parity == 0:
                    nc.scalar.activation(dst_ap, src_ap, Relu, bias=bias_ap)
                else:
                    nc.vector.tensor_scalar(dst_ap, src_ap, bias_ap, 0.0, Add, Max)
                parity ^= 1

            def strided(tile_t, j, b0, nb, c):
                v = tile_t[:].rearrange("p (i c) -> p i c", i=M)
                return v[:, b0:b0 + nb, off[j]:off[j] + c]

            # ---- L1: h1 = relu(W00[a0].T @ x0 + b00[a0])   (K=64, rows 0:64)
            def L1(i):
                for mo in range(2):
                    pt = ps.tile([128, 1024], F32, tag="ps")
                    ptb = pt[:].rearrange("p (b c) -> p b c", b=2)
                    for s in range(2):
                        nc.tensor.matmul(
                            ptb[:, s, 0:half],
                            w0zs[:, i * H + mo * 128: i * H + (mo + 1) * 128],
                            xts[:, i * Sp + s * half: i * Sp + (s + 1) * half],
                            start=True, stop=True)
                    dst_tile = h1a if mo == 0 else h1b
                    dst = dst_tile[:, i * Sp: i * Sp + S].rearrange("p (b c) -> p b c", b=2)
                    evict_relu(dst, ptb[:, :, 0:half], bss[:, mo * M + i: mo * M + i + 1])

            # ---- L2: h = relu(W01[a0].T @ h1 + b01[a0])   (K=256 -> 2 accum)
            def L2(i):
                pt = ps.tile([128, 1024], F32, tag="ps")
                ptb = pt[:].rearrange("p (b c) -> p b c", b=2)
                for s in range(2):
                    for kc, srcT in enumerate((h1a, h1b)):
                        nc.tensor.matmul(
                            ptb[:, s, 0:half],
                            w01s[:, (kc * M + i) * O0: (kc * M + i + 1) * O0],
                            srcT[:, i * Sp + s * half: i * Sp + (s + 1) * half],
                            start=(kc == 0), stop=(kc == 1))
                dst = hs[:, i * Sp: i * Sp + S].rearrange("p (b c) -> p b c", b=2)
                evict_relu(dst, ptb[:, :, 0:half], bss[:, 16 + i: 16 + i + 1])

            # ---- L3: g1 = relu(W1p[a1].T @ x1 + b1p[a1])  (K=64, rows 64:128)
            def L3(j):
                cj = Cj[j]
                xv = xts[:].rearrange("p (i c) -> p i c", i=M)
                for mo in range(2):
                    pt = ps.tile([128, 1024], F32, tag="ps")
                    ptb = pt[:].rearrange("p (b c) -> p b c", b=2)
                    for s in range(2):
                        nc.tensor.matmul(
                            ptb[:, s, 0:4 * cj].rearrange("p (i c) -> p i c", c=cj),
                            w1zs[:, j * H + mo * 128: j * H + (mo + 1) * 128],
                            xv[:, 4 * s:4 * s + 4, off[j]:off[j] + cj],
                            start=True, stop=True)
                    src_ap = ptb[:, :, 0:4 * cj].rearrange("p b (i c) -> p b i c", c=cj)
                    dst_tile = g1a if mo == 0 else g1b
                    evict_relu(strided(dst_tile, j, 0, 8, cj), src_ap,
                               bss[:, 24 + mo * M + j: 24 + mo * M + j + 1])

            # ---- L4: g = relu(W1a[a1].T @ concat(h, g1) + b1a[a1]) (K=384 -> 3 accum)
            def L4(j):
                cj = Cj[j]
                pt = ps.tile([128, 1024], F32, tag="ps")
                ptb = pt[:].rearrange("p (b c) -> p b c", b=2)
                for s in range(2):
                    ptv = ptb[:, s, 0:4 * cj].rearrange("p (i c) -> p i c", c=cj)
                    for kc, srcT in enumerate((hs, g1a, g1b)):
                        nc.tensor.matmul(
                            ptv,
                            wCs[:, (kc * M + j) * O0: (kc * M + j + 1) * O0],
                            strided(srcT, j, 4 * s, 4, cj),
                            start=(kc == 0), stop=(kc == 2))
                src_ap = ptb[:, :, 0:4 * cj].rearrange("p b (i c) -> p b i c", c=cj)
                evict_relu(strided(gs, j, 0, 8, cj), src_ap,
                           bss[:, 40 + j: 40 + j + 1])

            # ---- L5: out = W1o[a1].T @ g  (identity; bias added on host)
            # 4 modules land in partition quarters of one PSUM tile.
            def L5(j, ptq):
                r = j % 4
                ptb = ptq[:].rearrange("p (b c) -> p b c", b=2)
                for s in range(2):
                    out_ap = ptb[32 * r:32 * r + 32, s, 0:4 * Cmax].rearrange(
                        "p (i c) -> p i c", c=Cmax)
                    nc.tensor.matmul(
                        out_ap,
                        wCs[:, 3 * M * O0 + j * O1: 3 * M * O0 + (j + 1) * O1],
                        strided(gs, j, 4 * s, 4, Cmax),
                        start=True, stop=True,
                        tile_position=(0, 32 * r))

            def evict_quartet(q, ptq):
                ptb = ptq[:].rearrange("p (b c) -> p b c", b=2)
                dst = outs[q][:].rearrange("p (b c) -> p b c", b=2)
                if q == 0:
                    nc.scalar.copy(dst, ptb[:, :, 0:4 * Cmax])
                    nc.gpsimd.dma_start(outT[:, 0:Q], outs[q][:])
                else:
                    nc.vector.tensor_copy(dst, ptb[:, :, 0:4 * Cmax])
                    nc.sync.dma_start(outT[:, Q:2 * Q], outs[q][:])

            # ---- schedule: L1, then L2/L3 interleaved, then L4 with lagged L5
            for i in range(M):
                L1(i)
            for j in range(M):
                L2(j)
                L3(j)

            ptq = None
            for j in range(M):
                L4(j)
                if j >= 1:
                    jj = j - 1
                    if jj % 4 == 0:
                        ptq = psq.tile([128, 1024], F32, tag="q")
                    L5(jj, ptq)
                    if jj == 3:
                        evict_quartet(0, ptq)
            L5(7, ptq)
            evict_quartet(1, ptq)

    nc.compile()
    return nc


def _pack_inputs(inputs, core, xcol, S, Sp, Cj, off, Cmax):
    """Host-side packing of weights/biases/inputs into SBUF-image layouts."""
    import ml_dtypes
    bf = ml_dtypes.bfloat16
    f = lambda a: np.ascontiguousarray(a.astype(bf))
    g = lambda a: np.ascontiguousarray(a.astype(np.float32))
    W00 = inputs["W00"]; W01 = inputs["W01"]; W1p = inputs["W1p"]
    W1a = inputs["W1a"]; W1o = inputs["W1o"]

    w0z = np.zeros((128, M * H), dtype=np.float32)
    w0z[0:64] = W00.transpose(1, 0, 2).reshape(D0, M * H)
    w1z = np.zeros((128, M * H), dtype=np.float32)
    w1z[64:128] = W1p.transpose(1, 0, 2).reshape(D1, M * H)
    w01 = W01.reshape(M, 2, 128, O0).transpose(2, 1, 0, 3).reshape(128, 2 * M * O0)
    wc = np.concatenate([
        W1a.reshape(M, 3, 128, O0).transpose(2, 1, 0, 3).reshape(128, 3 * M * O0),
        W1o.transpose(1, 0, 2).reshape(128, M * O1)], axis=1)

    bs = np.zeros((128, 48), dtype=np.float32)
    bs[:, 0:16] = inputs["b00"].reshape(M, 2, 128).transpose(2, 1, 0).reshape(128, 16)
    bs[:, 16:24] = inputs["b01"].T
    bs[:, 24:40] = inputs["b1p"].reshape(M, 2, 128).transpose(2, 1, 0).reshape(128, 16)
    bs[:, 40:48] = inputs["b1a"].T

    input_val = inputs["input_val"]
    feat0 = input_val[:, :D0].astype(bf)
    feat1 = input_val[:, D0:D0 + D1].astype(bf)
    ncolp = M * Sp
    XT = np.zeros((NCORES, 128, ncolp), dtype=bf)
    XT[core, 0:64, xcol] = feat0
    XT[core, 64:128, xcol] = feat1

    return {"w0z": f(w0z), "w1z": f(w1z), "w01": f(w01), "wc": f(wc), "bs": g(bs)}, XT


def _route(input_val):
    """Assign each sample to a (core, column) in the blocked layout."""
    a0 = np.argmax(input_val[:, D0 + D1: D0 + D1 + M], axis=1)
    a1 = np.argmax(input_val[:, D0 + D1 + M: D0 + D1 + 2 * M], axis=1)
    B = input_val.shape[0]
    nij = np.zeros((M, M), dtype=np.int64)
    np.add.at(nij, (a0, a1), 1)
    # capacities rounded up to even, min 64
    Cj = np.maximum((-(-nij.max(axis=0) // NCORES) + 1) // 2 * 2, 64)
    off = np.concatenate([[0], np.cumsum(Cj)[:-1]]).astype(np.int64)
    S = int(Cj.sum())
    Cmax = int(Cj.max())
    # per-module stride padded so L5's uniform Cmax-wide reads stay in bounds
    pad = max(0, Cmax + int(off[-1]) - S)
    Sp = S + ((pad + 7) // 8) * 8

    pairkey = a0 * M + a1
    order = np.argsort(pairkey, kind="stable")
    counts = np.bincount(pairkey, minlength=M * M)
    group_start = np.concatenate([[0], np.cumsum(counts)[:-1]])
    rank_sorted = np.arange(B) - np.repeat(group_start, counts)
    rank = np.empty(B, dtype=np.int64)
    rank[order] = rank_sorted
    core = rank % NCORES
    slot = rank // NCORES
    assert np.all(slot < Cj[a1]), "capacity overflow"
    xcol = a0 * Sp + off[a1] + slot
    return (core, xcol, a0, a1, slot, S, Sp,
            [int(c) for c in Cj], [int(o) for o in off], Cmax)


def kernel(**inputs):
    import os
    from concourse.bass_utils import run_bass_kernel_spmd

    inputs = {k: np.asarray(v, dtype=np.float32) for k, v in inputs.items()}
    input_val = inputs["input_val"]

    core, xcol, a0, a1, slot, S, Sp, Cj, off, Cmax = _route(input_val)
    wmap, XT = _pack_inputs(inputs, core, xcol, S, Sp, Cj, off, Cmax)

    nc = _build_bass(S, Sp, Cj, off, Cmax)

    in_maps = [dict(wmap, xt=np.ascontiguousarray(XT[c])) for c in range(NCORES)]
    res = run_bass_kernel_spmd(nc, in_maps, core_ids=list(range(NCORES)),
                               tmpdir=os.environ.get("BASS_TMPDIR"))
    global _LAST_RESULTS
    _LAST_RESULTS = res

    Q = 2 * 4 * Cmax
    OUT = np.stack([r["outT"] for r in res.results]).astype(np.float32)  # [NC, 128, 2Q]
    rows = 32 * (a1 % 4)
    cols = (a1 // 4) * Q + (a0 // 4) * (4 * Cmax) + (a0 % 4) * Cmax + slot
    out = OUT[core[:, None], rows[:, None] + np.arange(O1)[None, :], cols[:, None]]
    out = out + inputs["b1o"][a1]
    return np.ascontiguousarray(out).astype(np.float32)
